# revision 24
# baseline (speedup 1.0000x reference)
"""Causal self-attention (B=4, T=2048, C=1024, H=16, D=64) on 8 trn2 NeuronCores.

Sharding: core c = (batch b = c//2, head-group g = c%2). Megatron-style within a
batch: each core computes 8 heads' q/k/v (column-parallel) and a row-parallel
partial out-projection. Host sums the two partials per batch and adds the
rank-1 bias term (bo + bv @ wo) -- valid because softmax rows sum to 1, so v's
bias never needs to enter the kernel.

Schedule: PE-stream discipline. Per k-tile the two score matmuls run at
alternating PE row groups (K=64 halves execute concurrently), the attention*V
pair follows as accumulation-chain links (weight loads hide inside chains),
and projection chains drop in between. q/k projections are fp8e4m3 DoubleRow
chains (256-deep contraction per instruction, 2x throughput; ~1.2e-2 rel err,
within the 2e-2 gate). The 1/Z normalize pipeline is split so no engine queue
ever stalls on DMA latency: Z-gather at unit end, reciprocal+DRAM-broadcast a
unit later, the O^T scale two units later. Late out-proj batches are held back
to cover the final unit's 1/Z round-trip; PE warm-up matmuls run under the
startup DMAs.
"""
import numpy as np
import ml_dtypes
from collections import deque

import concourse.tile as tile
from concourse import bacc, mybir
from concourse.bass_utils import run_bass_kernel_spmd

BF16 = ml_dtypes.bfloat16
FP8 = ml_dtypes.float8_e4m3
F32 = mybir.dt.float32
F16 = mybir.dt.float16
BT16 = mybir.dt.bfloat16
F8 = mybir.dt.float8e4
AF = mybir.ActivationFunctionType
ALU = mybir.AluOpType
PM = mybir.MatmulPerfMode

B, T, C, H, D = 4, 2048, 1024, 16, 64
G = 2              # head groups (cores per batch)
HL = H // G        # heads per core = 8
HD = HL * D        # local head dims = 512
NP = 4             # head pairs per core
NJQ = T // 512     # q chunks of 512 = 4
NIK = T // 128     # k tiles of 128 = 16
KC = C // 128      # contraction chunks = 8
KP = KC // 2       # fp8 DoubleRow chunk pairs = 4

QKPROJ_FP8 = True
QK_SWIL = False
AV_HALVES = False

_CACHED = {}


def _build():
    nc = bacc.Bacc("TRN2", debug=False)
    # host-prearranged layouts: per-partition-contiguous (2-8 KB runs, full DMA
    # BW). wq8/wk8 hold DoubleRow slab pairs contiguously: [p, t, pair, 2, 128].
    xp = nc.dram_tensor("xp", [NJQ, 128, KC * 512], BT16, kind="ExternalInput").ap()
    if QKPROJ_FP8:
        x8p = nc.dram_tensor("x8p", [NJQ, 128, KC * 512], F8, kind="ExternalInput").ap()
        wqk8 = nc.dram_tensor("wqk8", [128, NP, 2 * KP * 256], F8, kind="ExternalInput").ap()
    else:
        wqkp = nc.dram_tensor("wqkp", [NP, 128, KC * 256], BT16, kind="ExternalInput").ap()
    wvp = nc.dram_tensor("wvp", [128, KC * 512], BT16, kind="ExternalInput").ap()
    wo = nc.dram_tensor("wo", [HD, C], BT16, kind="ExternalInput").ap()
    bq = nc.dram_tensor("bq", [128, NP], F32, kind="ExternalInput").ap()
    bk = nc.dram_tensor("bk", [128, NP], F32, kind="ExternalInput").ap()
    masks = nc.dram_tensor("masks", [128, 4, 512], BT16, kind="ExternalInput").ap()
    rcp_dram = nc.dram_tensor("rcp_dram", [NJQ, 8, 512], BT16).ap()
    y = nc.dram_tensor("y", [T, C], BT16, kind="ExternalOutput").ap()

    with tile.TileContext(nc) as tc:
        with (
            tc.tile_pool(name="consts", bufs=1) as consts,
            tc.tile_pool(name="xt", bufs=2) as xtp,
            tc.tile_pool(name="qk", bufs=1) as qkp,
            tc.tile_pool(name="vp", bufs=1) as vp,
            tc.tile_pool(name="otp", bufs=1) as otp,
            tc.tile_pool(name="pt", bufs=8) as ptp,
            tc.tile_pool(name="ptmp", bufs=4) as ptmpp,
            tc.tile_pool(name="zn", bufs=3) as znp,
            tc.tile_pool(name="yst", bufs=4) as ystp,
            tc.tile_pool(name="ps", bufs=2, space="PSUM") as ps,
        ):
            # ---- constants ----
            bq_sb = consts.tile([128, NP], F32, tag="bq")
            bk_sb = consts.tile([128, NP], F32, tag="bk")
            bq_sb2, bk_sb2 = bq_sb, bk_sb
            if not QKPROJ_FP8:
                nc.sync.dma_start(bq_sb, bq)
                nc.sync.dma_start(bk_sb, bk)
            masks_sb = consts.tile([128, 4, 512], BT16, tag="masks")

            if QKPROJ_FP8:
                # combined q+k weights per head-pair: one DMA per t
                wqk8_t = [consts.tile([128, 2, KP, 2, 128], F8, tag=f"wqk8{t}",
                                      name=f"wqk8{t}") for t in range(NP)]
                wqk8_w = [wqk8[:, t].rearrange("p (w j s c) -> p w j s c",
                                               w=2, s=2, c=128)
                          for t in range(NP)]
                x8ts = {}
            else:
                wqk_t = [consts.tile([128, KC, 256], BT16, tag=f"wqk{t}",
                                     name=f"wqk{t}") for t in range(NP)]
                wqk_w = [wqkp[t].rearrange("p (k c) -> p k c", c=256)
                         for t in range(NP)]

            def dma_xt(jt):
                # bf16 x for the v-projection; mid-kernel chunks keep off the
                # sync queue (reserved for 1/Z chains). jt==0 loads in column
                # slices matching v_unit consumption order (stationary slices
                # span all KC chunks).
                xt = xtp.tile([128, KC, 512], BT16, tag="xt", bufs=3, name=f"xt{jt}")
                xv = xp[jt].rearrange("p (k c) -> p k c", c=512)
                if jt == 0:
                    for s in range(4):
                        cs = slice(s * 128, (s + 1) * 128)
                        nc.gpsimd.dma_start(xt[:, :, cs], xv[:, :, cs])
                else:
                    nc.gpsimd.dma_start(xt[:, 0:4, :], xv[:, 0:4, :])
                    nc.gpsimd.dma_start(xt[:, 4:8, :], xv[:, 4:8, :])
                return xt

            def dma_xt8(jt, queue):
                x8t = xtp.tile([128, KC, 512], F8, tag="xt8", bufs=3, name=f"x8t{jt}")
                x8v = x8p[jt].rearrange("p (k c) -> p k c", c=512)
                queue.dma_start(x8t[:, 0:4, :], x8v[:, 0:4, :])
                queue.dma_start(x8t[:, 4:8, :], x8v[:, 4:8, :])
                return x8t

            # Startup: the critical loads (pair-0 q/k weights + fp8 x chunk 0)
            # stream on the sync queue in consumption order; the gpsimd queue
            # is stalled behind a copy that depends on the last fp8 pair so
            # bulk loads don't contend with the critical path.
            dummy_in = consts.tile([1, 16], F32, tag="dummy_in")
            nc.vector.memset(dummy_in, 0.0)
            dummy_out = consts.tile([1, 16], F32, tag="dummy_out")
            nc.scalar.activation(dummy_out, dummy_in, AF.Exp)

            # PE warm-up: dummy matmuls with no DMA deps run during the
            # startup loads, pulling the tensor engine out of its low p-state
            # before real work lands.
            warm_w = consts.tile([64, 16], BT16, tag="warm_w")
            nc.vector.memset(warm_w, 0.5)
            warm_x = consts.tile([64, 512], BT16, tag="warm_x")
            nc.vector.memset(warm_x, 0.5)
            warm_ps = ps.tile([16, 512], F32, tag="fp", bufs=2, name="warm_ps")
            for wi in range(8):
                nc.tensor.matmul(warm_ps, warm_w, warm_x, start=True, stop=True)

            if QKPROJ_FP8:
                x8t0 = xtp.tile([128, KC, 512], F8, tag="xt8", bufs=3, name="x8t0")
                x8v0 = x8p[0].rearrange("p (k c) -> p k c", c=512)
                nc.sync.dma_start(wqk8_t[0][:, :, 0:2], wqk8_w[0][:, :, 0:2])
                nc.sync.dma_start(x8t0[:, 0:4, :], x8v0[:, 0:4, :])
                nc.sync.dma_start(wqk8_t[0][:, :, 2:4], wqk8_w[0][:, :, 2:4])
                nc.sync.dma_start(x8t0[:, 4:8, :], x8v0[:, 4:8, :])
                nc.sync.dma_start(wqk8_t[1], wqk8_w[1])
                nc.sync.dma_start(bq_sb2, bq)
                nc.sync.dma_start(bk_sb2, bk)
                nc.sync.dma_start(masks_sb[:, 0:1, :], masks[:, 0:1, :])
                nc.sync.dma_start(wqk8_t[2], wqk8_w[2])
                nc.sync.dma_start(masks_sb[:, 1:4, :], masks[:, 1:4, :])
                nc.sync.dma_start(wqk8_t[3], wqk8_w[3])
                x8ts[0] = x8t0
                # gpsimd stall: depends on the critical fp8 x chunk
                dummy_gp = consts.tile([1, 8], F8, tag="dummy_gp")
                nc.gpsimd.tensor_copy(dummy_gp, x8t0[0:1, 6, 0:8])
                xts = {0: dma_xt(0)}
            else:
                xt0 = xtp.tile([128, KC, 512], BT16, tag="xt", bufs=3, name="xt0")
                xv0 = xp[0].rearrange("p (k c) -> p k c", c=512)
                nc.sync.dma_start(xt0[:, 0:2, :], xv0[:, 0:2, :])
                nc.sync.dma_start(wqk_t[0], wqk_w[0])
                nc.sync.dma_start(xt0[:, 2:4, :], xv0[:, 2:4, :])
                nc.sync.dma_start(xt0[:, 4:6, :], xv0[:, 4:6, :])
                nc.sync.dma_start(xt0[:, 6:8, :], xv0[:, 6:8, :])
                nc.sync.dma_start(wqk_t[1], wqk_w[1])
                nc.sync.dma_start(masks_sb, masks)
                nc.sync.dma_start(wqk_t[2], wqk_w[2])
                nc.sync.dma_start(wqk_t[3], wqk_w[3])
                dummy_gp = consts.tile([1, 8], BT16, tag="dummy_gp")
                nc.gpsimd.tensor_copy(dummy_gp, xt0[0:1, 6, 0:8])
                xts = {0: xt0}

            # wv on the scalar queue, stalled behind the mid-critical fp8 x
            # pair so it overlaps (not contends with) the startup loads
            wv_sb = consts.tile([128, KC, HD], BT16, tag="wv")
            wv_v = wvp.rearrange("p (k c) -> p k c", c=512)
            dummy_sc = consts.tile([1, 8], F8 if QKPROJ_FP8 else BT16, tag="dummy_sc")
            if QKPROJ_FP8:
                nc.scalar.copy(dummy_sc, x8ts[0][0:1, 2, 0:8])
            else:
                nc.scalar.copy(dummy_sc, xts[0][0:1, 2, 0:8])
            nc.scalar.dma_start(wv_sb[:, 0:4, :], wv_v[:, 0:4, :])
            nc.scalar.dma_start(wv_sb[:, 4:8, :], wv_v[:, 4:8, :])
            wo_sb = consts.tile([128, NP, C], BT16, tag="wo")

            # ---- persistent activations ----
            qT = [qkp.tile([128, T], BT16, tag=f"qT{t}", name=f"qT{t}") for t in range(NP)]
            kT = [qkp.tile([128, T], BT16, tag=f"kT{t}", name=f"kT{t}") for t in range(NP)]
            v_sb = [vp.tile([128, HL * 65], BT16, tag=f"v{i}", name=f"v{i}") for i in range(NIK)]
            oT = [otp.tile([128, T], BT16, tag=f"oT{t}", name=f"oT{t}") for t in range(NP)]

            # ---- phase-1 units ----
            def qk_q(jt, t, xt):
                p = ps.tile([128, 512], F32, tag="fp", bufs=2, name=f"pq{jt}_{t}")
                if QKPROJ_FP8:
                    for j in range(KP):
                        nc.tensor.matmul(
                            p, wqk8_t[t][:, 0, j], xt[:, 2*j:2*j+2, :],
                            start=(j == 0), stop=(j == KP - 1),
                            perf_mode=(PM.DoubleRowSwInterleave if QK_SWIL
                                       else PM.DoubleRow),
                        )
                else:
                    for k in range(KC):
                        nc.tensor.matmul(
                            p, wqk_t[t][:, k, 0:128], xt[:, k, :],
                            start=(k == 0), stop=(k == KC - 1),
                        )
                nc.vector.tensor_scalar(
                    qT[t][:, jt * 512:(jt + 1) * 512], p,
                    0.125, bq_sb[:, t:t + 1], ALU.mult, ALU.add,
                )

            def qk_k(jt, t, xt):
                p = ps.tile([128, 512], F32, tag="fp", bufs=2, name=f"pk{jt}_{t}")
                if QKPROJ_FP8:
                    for j in range(KP):
                        nc.tensor.matmul(
                            p, wqk8_t[t][:, 1, j], xt[:, 2*j:2*j+2, :],
                            start=(j == 0), stop=(j == KP - 1),
                            perf_mode=(PM.DoubleRowSwInterleave if QK_SWIL
                                       else PM.DoubleRow),
                        )
                else:
                    for k in range(KC):
                        nc.tensor.matmul(
                            p, wqk_t[t][:, k, 128:256], xt[:, k, :],
                            start=(k == 0), stop=(k == KC - 1),
                        )
                nc.vector.tensor_scalar_add(
                    kT[t][:, jt * 512:(jt + 1) * 512], p, bk_sb[:, t:t + 1]
                )

            def v_unit(jt, s, xt):
                ik = jt * 4 + s
                p = ps.tile([128, 512], F32, tag="fp", bufs=2, name=f"pv{ik}")
                for k in range(KC):
                    nc.tensor.matmul(
                        p, xt[:, k, s * 128:(s + 1) * 128], wv_sb[:, k, :],
                        start=(k == 0), stop=(k == KC - 1),
                    )
                vg = v_sb[ik].rearrange("p (h c) -> p h c", c=65)
                nc.vector.tensor_copy(
                    vg[:, :, 0:64], p.rearrange("p (h c) -> p h c", c=64)
                )
                nc.gpsimd.memset(vg[:, :, 64:65], 1.0)

            # ---- out-projection (m, n) sub-chunk; one merged y DMA per m ----
            ys_tiles = {}

            def phase3_n(m, n, alt=False):
                p = ps.tile([128, 512], F32, tag="fp", bufs=2, name=f"py{m}_{n}")
                for t in range(NP):
                    nc.tensor.matmul(
                        p, oT[t][:, m * 128:(m + 1) * 128],
                        wo_sb[:, t, n * 512:(n + 1) * 512],
                        start=(t == 0), stop=(t == NP - 1),
                    )
                if n == 0:
                    ys_tiles[m] = ystp.tile([128, 1024], BT16, tag="y", name=f"ys{m}")
                ys = ys_tiles[m]
                if alt:
                    # tail: per-half writes on alternating queues so the final
                    # transfers start as soon as each half is evicted
                    nc.scalar.copy(ys[:, n * 512:(n + 1) * 512], p)
                    eng = nc.sync if m % 2 == 1 else nc.gpsimd
                    eng.dma_start(
                        y[m * 128:(m + 1) * 128, n * 512:(n + 1) * 512],
                        ys[:, n * 512:(n + 1) * 512],
                    )
                else:
                    nc.vector.tensor_copy(ys[:, n * 512:(n + 1) * 512], p)
                    if n == 1:
                        nc.gpsimd.dma_start(y[m * 128:(m + 1) * 128, :], ys)

            # ---- attention ----
            def st_block(t, jq, ik, pts):
                d = ik - 4 * jq
                c0 = 128 * d if d > 0 else 0   # first potentially-valid column
                st = ps.tile([128, 1024], F32, tag="st", name=f"st{t}_{jq}_{ik}")
                stg = st.rearrange("p (h q) -> p h q", q=512)
                for hh in range(2):
                    r = slice(hh * 64, hh * 64 + 64)
                    nc.tensor.matmul(
                        stg[:, hh, c0:512],
                        kT[t][r, ik * 128:(ik + 1) * 128],
                        qT[t][r, jq * 512 + c0:(jq + 1) * 512],
                        start=True, stop=True,
                    )
                pt = ptp.tile([128, 1024], BT16, tag="pt", name=f"pt{t}_{jq}_{ik}")
                ptg = pt.rearrange("p (h q) -> p h q", q=512)
                if d >= 0:
                    ptm = ptmpp.tile([128, 1024], BT16, tag="ptmp", name=f"ptm{t}_{jq}_{ik}")
                    ptmg = ptm.rearrange("p (h q) -> p h q", q=512)
                    nc.scalar.activation(ptmg[:, :, c0:512], stg[:, :, c0:512], AF.Exp)
                    for hh in range(2):
                        nc.vector.tensor_mul(
                            ptg[:, hh, c0:512],
                            ptmg[:, hh, c0:512],
                            masks_sb[:, d, c0:512],
                        )
                else:
                    nc.scalar.activation(pt, st, AF.Exp)
                pts[ik] = (pt, c0)

            def av(t, ik, nik, pts, o_ps):
                # split the K=128 contraction into row-group halves so every
                # attention matmul is a half-array op: the two groups stream
                # concurrently. Both halves accumulate into the same PSUM
                # region (FIFO start order makes the start=True half land
                # first at every address).
                pt, c0 = pts[ik]
                ptg = pt.rearrange("p (h q) -> p h q", q=512)
                for hh in range(2):
                    h = 2 * t + hh
                    if AV_HALVES:
                        for ha in range(2):
                            r = slice(64 * ha, 64 * ha + 64)
                            nc.tensor.matmul(
                                o_ps[hh][:, c0:512],
                                v_sb[ik][r, h * 65:h * 65 + 65],
                                ptg[r, hh, c0:512],
                                start=(ik == 0 and ha == 0),
                                stop=(ik == nik - 1 and ha == 1),
                                skip_group_check=True,
                            )
                    else:
                        nc.tensor.matmul(
                            o_ps[hh][:, c0:512], v_sb[ik][:, h * 65:h * 65 + 65],
                            ptg[:, hh, c0:512],
                            start=(ik == 0), stop=(ik == nik - 1),
                        )

            def attention(t, jq, fills, evict_split=False):
                nik = 4 * jq + 4
                o_ps = [
                    ps.tile([65, 512], F32, tag="ot", bufs=2, name=f"ops{t}_{jq}_{_h}")
                    for _h in range(2)
                ]
                pts = {}
                # homogeneous bursts: 4 st matmuls (2 ik x 2 row groups), one
                # fill chain, then 4 av chain matmuls for the previous pair
                for ik in range(nik):
                    st_block(t, jq, ik, pts)
                    if (jq == 0 and ik % 2 == 1) or (jq > 0 and ik % 4 == 3):
                        for _ in range(2):
                            if fills:
                                fills.popleft()()
                    if ik > 0:
                        av(t, ik - 1, nik, pts, o_ps)
                av(t, nik - 1, nik, pts, o_ps)
                # evict Z row + unnormalized O^T, freeing the PSUM accumulators.
                # Last unit: Z rows first so the tail 1/Z chain starts早.
                out_h = []
                if evict_split:
                    for hh in range(2):
                        ouz = znp.tile([65, 512], F32, tag="ouz", bufs=6, name=f"oz{t}_{jq}_{hh}")
                        nc.vector.tensor_copy(ouz[64:65, :], o_ps[hh][64:65, :])
                        out_h.append(ouz)
                    for hh in range(2):
                        nc.vector.tensor_copy(out_h[hh][0:64, :], o_ps[hh][0:64, :])
                else:
                    for hh in range(2):
                        ouz = znp.tile([65, 512], F32, tag="ouz", bufs=6, name=f"oz{t}_{jq}_{hh}")
                        nc.vector.tensor_copy(ouz, o_ps[hh])
                        out_h.append(ouz)
                while fills:
                    fills.popleft()()
                return out_h

            import concourse.bass as bass_mod

            def normalize_a1(t, jq, evicted):
                # Stage A1: gather both heads' Z rows [1,512] as [8,64] each
                # into one [16,64] tile. DMA only -- the dependent reciprocal
                # runs a unit later so no engine queue stalls on this latency.
                zb = znp.tile([16, 64], F32, tag="zb", bufs=3, name=f"zb{t}_{jq}")
                for hh in range(2):
                    ouz = evicted[hh]
                    nc.sync.dma_start(
                        zb[8 * hh:8 * hh + 8, :],
                        ouz[64:65, :].rearrange("o (p q) -> o p q", p=8),
                    )
                return zb

            def normalize_a2(t, jq, zb):
                # Stage A2: reciprocal, then broadcast 1/Z via a DRAM
                # round-trip (partition-step-0 DMA reads are legal from DRAM).
                rcp = znp.tile([16, 64], F32, tag="rcpb", bufs=2, name=f"rcp{t}_{jq}")
                nc.vector.reciprocal(rcp, zb)
                rcp16 = znp.tile([16, 64], BT16, tag="rcp16b", bufs=2, name=f"rcp16{t}_{jq}")
                nc.vector.tensor_copy(rcp16, rcp)
                nc.sync.dma_start(
                    rcp_dram[jq, 2 * t:2 * t + 2, :].rearrange("h (p q) -> (h p) q", p=8),
                    rcp16,
                )
                bcs = []
                for hh in range(2):
                    bc_sb = znp.tile([64, 512], BT16, tag="bc_sb", bufs=6, name=f"bs{t}_{jq}_{hh}")
                    src = rcp_dram[jq, 2 * t + hh, :]
                    bcast = bass_mod.AP(
                        tensor=src.tensor, offset=src.offset,
                        ap=[[0, 64]] + [list(a) for a in src.ap],
                    )
                    nc.sync.dma_start(bc_sb, bcast)
                    bcs.append(bc_sb)
                return bcs

            def normalize_b(t, jq, evicted, bcs):
                # Stage B (one slot later, after the broadcast landed): scale
                # O^T by 1/Z.
                qs2 = slice(jq * 512, (jq + 1) * 512)
                nc.vector.tensor_mul(oT[t][0:64, qs2], evicted[0][0:64, :], bcs[0])
                tmp = znp.tile([64, 512], BT16, tag="tmp_o", bufs=2, name=f"tm{t}_{jq}")
                nc.gpsimd.tensor_mul(tmp, evicted[1][0:64, :], bcs[1])
                nc.gpsimd.dma_start(oT[t][64:128, qs2], tmp)

            # ---- main schedule ----
            # unit u = jq*4 + t; prelude runs qk(0) and qk(1) so attention(u)
            # can fill qk(u+2) -- gives the v path two units of DMA headroom.
            def qk_unit(u):
                jt, tt = divmod(u, NP)
                xx = x8ts[jt] if QKPROJ_FP8 else xts[jt]
                qk_q(jt, tt, xx)
                qk_k(jt, tt, xx)

            pendA2 = deque()  # awaiting stage-A2 (rcp + broadcast), next unit
            pendB = deque()   # awaiting stage-B (muls), one slot later
            p3q = deque()
            qk_unit(0)
            if QKPROJ_FP8:
                x8ts[1] = dma_xt8(1, nc.sync)
            qk_unit(1)
            xts[1] = dma_xt(1)
            nc.gpsimd.dma_start(wo_sb, wo.rearrange("(t p) c -> p t c", p=128))
            last_ev = None
            for jq in range(NJQ):
                for t in range(NP):
                    u = jq * 4 + t
                    if t == 1 and jq >= 1 and jq + 1 < NJQ:
                        xts[jq + 1] = dma_xt(jq + 1)
                        if QKPROJ_FP8:
                            x8ts[jq + 1] = dma_xt8(jq + 1, nc.sync)
                    fills = deque()
                    if jq == 0 and t == 0:
                        for s in range(4):
                            fills.append(lambda ss=s: v_unit(0, ss, xts[0]))
                    if u + 2 < NJQ * NP:
                        fills.append(lambda uu=u + 2: qk_unit(uu))
                    # hold the late out-proj batches in p3q for the tail (they
                    # cover the 1/Z DRAM round-trip latency there)
                    if jq != NJQ - 1 and not (jq == NJQ - 2 and t >= 2):
                        for _ in range(2 if t == NP - 1 else 1):
                            if p3q:
                                m = p3q.popleft()
                                fills.append(lambda mm=m: phase3_n(mm, 0))
                                fills.append(lambda mm=m: phase3_n(mm, 1))
                    if t >= 2 and jq + 1 < NJQ:
                        s0 = 2 * (t - 2)
                        fills.append(lambda jt=jq + 1, s=s0: v_unit(jt, s, xts[jt]))
                        fills.append(lambda jt=jq + 1, s=s0 + 1: v_unit(jt, s, xts[jt]))
                    ev = attention(t, jq, fills,
                                   evict_split=(jq == NJQ - 1 and t == NP - 1))
                    if not (jq == NJQ - 1 and t == NP - 1):
                        zb = normalize_a1(t, jq, ev)
                        pendA2.append((t, jq, ev, zb))
                    while len(pendA2) >= 2:
                        ta2, ja2, eva2, zba2 = pendA2.popleft()
                        bcs = normalize_a2(ta2, ja2, zba2)
                        pendB.append((ta2, ja2, eva2, bcs))
                    nb_thresh = 2
                    while len(pendB) >= nb_thresh:
                        tb, jb, evb, bcsb = pendB.popleft()
                        normalize_b(tb, jb, evb, bcsb)
                        if tb == NP - 1:
                            p3q.extend(range(4 * jb, 4 * jb + 4))
                    last_ev = ev

            # ---- tail ----
            # (3,3): same DRAM-broadcast 1/Z path as every other unit. The
            # deferred out-proj chains in p3q keep the PE busy while the
            # broadcast lands; the final out-proj chains follow per m-chunk.
            ta, ja = NP - 1, NJQ - 1
            eva = last_ev
            zb_t = normalize_a1(ta, ja, eva)
            while pendA2:
                ta2, ja2, eva2, zba2 = pendA2.popleft()
                pendB.append((ta2, ja2, eva2, normalize_a2(ta2, ja2, zba2)))
            while pendB:
                tb, jb, evb, bcsb = pendB.popleft()
                normalize_b(tb, jb, evb, bcsb)
                if tb == NP - 1:
                    p3q.extend(range(4 * jb, 4 * jb + 4))
            bcs_t = normalize_a2(ta, ja, zb_t)
            # Final m-batch out-proj split: closed t=0..2 partial chains (no
            # dependency on the last 1/Z) run during the broadcast latency and
            # evict to fp16 SBUF; after the normalize muls only a single t=3
            # matmul plus a DVE add per chain remains.
            parts = {}
            for mi in range(4):
                m = 4 * ja + mi
                for n in range(2):
                    fp_t = ps.tile([128, 512], F32, tag="fp", bufs=2,
                                   name=f"pp{m}_{n}")
                    for t in range(NP - 1):
                        nc.tensor.matmul(
                            fp_t, oT[t][:, m * 128:(m + 1) * 128],
                            wo_sb[:, t, n * 512:(n + 1) * 512],
                            start=(t == 0), stop=(t == NP - 2),
                        )
                    part = znp.tile([128, 512], F16, tag="part", bufs=8,
                                    name=f"part{m}_{n}")
                    nc.vector.tensor_copy(part, fp_t)
                    parts[(m, n)] = part
            while p3q:     # deferred chains also fill the 1/Z window
                m = p3q.popleft()
                phase3_n(m, 0)
                phase3_n(m, 1)
            for mi in range(4):
                cs = slice(mi * 128, (mi + 1) * 128)
                gs = slice(ja * 512 + mi * 128, ja * 512 + (mi + 1) * 128)
                nc.vector.tensor_mul(oT[ta][0:64, gs], eva[0][0:64, cs],
                                     bcs_t[0][:, cs])
                tmp = znp.tile([64, 128], BT16, tag="tmp_os", bufs=4, name=f"tms{mi}")
                nc.gpsimd.tensor_mul(tmp, eva[1][0:64, cs], bcs_t[1][:, cs])
                nc.sync.dma_start(oT[ta][64:128, gs], tmp)
            for mi in range(4):
                m = 4 * ja + mi
                ys = ystp.tile([128, 1024], BT16, tag="y", name=f"ysF{m}")
                for n in range(2):
                    t3 = ps.tile([128, 512], F32, tag="fp", bufs=2,
                                 name=f"t3{m}_{n}")
                    nc.tensor.matmul(
                        t3, oT[ta][:, m * 128:(m + 1) * 128],
                        wo_sb[:, ta, n * 512:(n + 1) * 512],
                        start=True, stop=True,
                    )
                    nc.vector.tensor_add(ys[:, n * 512:(n + 1) * 512], t3,
                                         parts[(m, n)])
                    eng = nc.sync if mi % 2 == 1 else nc.gpsimd
                    eng.dma_start(
                        y[m * 128:(m + 1) * 128, n * 512:(n + 1) * 512],
                        ys[:, n * 512:(n + 1) * 512],
                    )

    nc.compile()
    return nc


def _host_prep(x, wq, bq, wk, bk, wv, wo):
    masks_np = np.zeros((128, 4, 512), dtype=BF16)
    qn = np.arange(512)[None, :]
    kn = np.arange(128)[:, None]
    for d in range(4):
        masks_np[:, d, :] = (qn >= kn + 128 * d).astype(BF16)

    per_g = []
    for g in range(G):
        cs = slice(g * HD, (g + 1) * HD)
        wv_g = wv[:, cs]                                            # [C, 512]
        wvp = wv_g.reshape(KC, 128, 512).transpose(1, 0, 2).reshape(128, KC * 512)
        m = {
            "wvp": np.ascontiguousarray(wvp).astype(BF16),
            "wo": np.ascontiguousarray(wo[cs, :]).astype(BF16),
            "bq": np.ascontiguousarray((bq[cs] / 8.0).reshape(NP, 128).T).astype(np.float32),
            "bk": np.ascontiguousarray(bk[cs].reshape(NP, 128).T).astype(np.float32),
            "masks": masks_np,
        }
        if QKPROJ_FP8:
            # [p, t, pair, slab, m]: element = w[(2*pair+slab)*128 + p, g*HD + t*128 + m]
            def pack8(w):
                wg = w[:, cs].reshape(KC, 128, NP, 128)       # [k, p, t, m]
                wg = wg.reshape(KP, 2, 128, NP, 128)          # [pair, slab, p, t, m]
                wg = wg.transpose(2, 3, 0, 1, 4)              # [p, t, pair, slab, m]
                if QK_SWIL:
                    # HW SwInterleave layout: stored[:, 2c+s] = logical[:, s, 127-c]
                    rev = wg[..., ::-1]                       # [p,t,pair,s,c]
                    wg = rev.transpose(0, 1, 2, 4, 3)         # [p,t,pair,c,s]
                return np.ascontiguousarray(
                    wg.reshape(128, NP, KP * 256)).astype(FP8)
            m["wqk8"] = np.ascontiguousarray(np.stack(
                [pack8(wq), pack8(wk)], axis=2).reshape(128, NP, 2 * KP * 256))
        else:
            wqkp = np.empty((NP, 128, KC * 256), dtype=np.float32)
            for t in range(NP):
                ts_ = slice(g * HD + t * 128, g * HD + (t + 1) * 128)
                blk = np.concatenate([wq[:, ts_], wk[:, ts_]], axis=1)  # [C, 256]
                wqkp[t] = blk.reshape(KC, 128, 256).transpose(1, 0, 2).reshape(128, KC * 256)
            m["wqkp"] = np.ascontiguousarray(wqkp).astype(BF16)
        per_g.append(m)
    in_maps = []
    xps = []
    x8ps = []
    for b in range(B):
        xT = x[b].T                                                 # [C, T]
        xpb = (xT.reshape(KC, 128, NJQ, 512).transpose(2, 1, 0, 3)
               .reshape(NJQ, 128, KC * 512))
        xpb = np.ascontiguousarray(xpb)
        xps.append(xpb.astype(BF16))
        if QKPROJ_FP8:
            x8ps.append(xpb.astype(FP8))
    for c in range(8):
        b, g = divmod(c, G)
        m = dict(per_g[g])
        m["xp"] = xps[b]
        if QKPROJ_FP8:
            m["x8p"] = x8ps[b]
        in_maps.append(m)
    return in_maps


def kernel(x, wq, bq, wk, bk, wv, bv, wo, bo):
    x = np.asarray(x, dtype=np.float32)
    wq = np.asarray(wq, dtype=np.float32)
    bq = np.asarray(bq, dtype=np.float32)
    wk = np.asarray(wk, dtype=np.float32)
    bk = np.asarray(bk, dtype=np.float32)
    wv = np.asarray(wv, dtype=np.float32)
    bv = np.asarray(bv, dtype=np.float32)
    wo = np.asarray(wo, dtype=np.float32)
    bo = np.asarray(bo, dtype=np.float32)

    if "nc" not in _CACHED:
        _CACHED["nc"] = _build()
    nc = _CACHED["nc"]

    in_maps = _host_prep(x, wq, bq, wk, bk, wv, wo)
    res = run_bass_kernel_spmd(nc, in_maps, core_ids=list(range(8)))

    const_row = (bo.astype(np.float64) + bv.astype(np.float64) @ wo.astype(np.float64))
    out = np.empty((B, T, C), dtype=np.float32)
    for b in range(B):
        acc = res.results[2 * b]["y"].astype(np.float64)
        acc += res.results[2 * b + 1]["y"].astype(np.float64)
        acc += const_row[None, :]
        out[b] = acc.astype(np.float32)
    return out


# revision 25
# speedup vs baseline: 1.0187x; 1.0187x over previous
"""Causal self-attention (B=4, T=2048, C=1024, H=16, D=64) on 8 trn2 NeuronCores.

Sharding: core c = (batch b = c//2, head-group g = c%2). Megatron-style within a
batch: each core computes 8 heads' q/k/v (column-parallel) and a row-parallel
partial out-projection. Host sums the two partials per batch and adds the
rank-1 bias term (bo + bv @ wo) -- valid because softmax rows sum to 1, so v's
bias never needs to enter the kernel.

Schedule: PE-stream discipline. Per k-tile the two score matmuls run at
alternating PE row groups (K=64 halves execute concurrently), the attention*V
pair follows as accumulation-chain links (weight loads hide inside chains),
and projection chains drop in between. q/k projections are fp8e4m3 DoubleRow
chains (256-deep contraction per instruction, 2x throughput; ~1.2e-2 rel err,
within the 2e-2 gate). The 1/Z normalize pipeline is split so no engine queue
ever stalls on DMA latency: Z-gather at unit end, reciprocal+DRAM-broadcast a
unit later, the O^T scale two units later. Late out-proj batches are held back
to cover the final unit's 1/Z round-trip; PE warm-up matmuls run under the
startup DMAs.
"""
import numpy as np
import ml_dtypes
from collections import deque

import concourse.tile as tile
from concourse import bacc, mybir
from concourse.bass_utils import run_bass_kernel_spmd

BF16 = ml_dtypes.bfloat16
FP8 = ml_dtypes.float8_e4m3
F32 = mybir.dt.float32
BT16 = mybir.dt.bfloat16
F8 = mybir.dt.float8e4
AF = mybir.ActivationFunctionType
ALU = mybir.AluOpType
PM = mybir.MatmulPerfMode

B, T, C, H, D = 4, 2048, 1024, 16, 64
G = 2              # head groups (cores per batch)
HL = H // G        # heads per core = 8
HD = HL * D        # local head dims = 512
NP = 4             # head pairs per core
NJQ = T // 512     # q chunks of 512 = 4
NIK = T // 128     # k tiles of 128 = 16
KC = C // 128      # contraction chunks = 8
KP = KC // 2       # fp8 DoubleRow chunk pairs = 4

QKPROJ_FP8 = True
QK_SWIL = False
AV_HALVES = False

_CACHED = {}


def _build():
    nc = bacc.Bacc("TRN2", debug=False)
    # host-prearranged layouts: per-partition-contiguous (2-8 KB runs, full DMA
    # BW). wq8/wk8 hold DoubleRow slab pairs contiguously: [p, t, pair, 2, 128].
    xp = nc.dram_tensor("xp", [NJQ, 128, KC * 512], BT16, kind="ExternalInput").ap()
    if QKPROJ_FP8:
        x8p = nc.dram_tensor("x8p", [NJQ, 128, KC * 512], F8, kind="ExternalInput").ap()
        wqk8 = nc.dram_tensor("wqk8", [128, NP, 2 * KP * 256], F8, kind="ExternalInput").ap()
    else:
        wqkp = nc.dram_tensor("wqkp", [NP, 128, KC * 256], BT16, kind="ExternalInput").ap()
    wvp = nc.dram_tensor("wvp", [128, KC * 512], BT16, kind="ExternalInput").ap()
    wo = nc.dram_tensor("wo", [HD, C], BT16, kind="ExternalInput").ap()
    bq = nc.dram_tensor("bq", [128, NP], F32, kind="ExternalInput").ap()
    bk = nc.dram_tensor("bk", [128, NP], F32, kind="ExternalInput").ap()
    masks = nc.dram_tensor("masks", [128, 4, 512], BT16, kind="ExternalInput").ap()
    rcp_dram = nc.dram_tensor("rcp_dram", [NJQ, 8, 512], BT16).ap()
    y = nc.dram_tensor("y", [T, C], BT16, kind="ExternalOutput").ap()

    with tile.TileContext(nc) as tc:
        with (
            tc.tile_pool(name="consts", bufs=1) as consts,
            tc.tile_pool(name="xt", bufs=2) as xtp,
            tc.tile_pool(name="qk", bufs=1) as qkp,
            tc.tile_pool(name="vp", bufs=1) as vp,
            tc.tile_pool(name="otp", bufs=1) as otp,
            tc.tile_pool(name="pt", bufs=8) as ptp,
            tc.tile_pool(name="ptmp", bufs=4) as ptmpp,
            tc.tile_pool(name="zn", bufs=3) as znp,
            tc.tile_pool(name="yst", bufs=4) as ystp,
            tc.tile_pool(name="ps", bufs=2, space="PSUM") as ps,
        ):
            # ---- constants ----
            bq_sb = consts.tile([128, NP], F32, tag="bq")
            bk_sb = consts.tile([128, NP], F32, tag="bk")
            bq_sb2, bk_sb2 = bq_sb, bk_sb
            if not QKPROJ_FP8:
                nc.sync.dma_start(bq_sb, bq)
                nc.sync.dma_start(bk_sb, bk)
            masks_sb = consts.tile([128, 4, 512], BT16, tag="masks")

            if QKPROJ_FP8:
                # combined q+k weights per head-pair: one DMA per t
                wqk8_t = [consts.tile([128, 2, KP, 2, 128], F8, tag=f"wqk8{t}",
                                      name=f"wqk8{t}") for t in range(NP)]
                wqk8_w = [wqk8[:, t].rearrange("p (w j s c) -> p w j s c",
                                               w=2, s=2, c=128)
                          for t in range(NP)]
                x8ts = {}
            else:
                wqk_t = [consts.tile([128, KC, 256], BT16, tag=f"wqk{t}",
                                     name=f"wqk{t}") for t in range(NP)]
                wqk_w = [wqkp[t].rearrange("p (k c) -> p k c", c=256)
                         for t in range(NP)]

            def dma_xt(jt):
                # bf16 x for the v-projection; mid-kernel chunks keep off the
                # sync queue (reserved for 1/Z chains). jt==0 loads in column
                # slices matching v_unit consumption order (stationary slices
                # span all KC chunks).
                xt = xtp.tile([128, KC, 512], BT16, tag="xt", bufs=3, name=f"xt{jt}")
                xv = xp[jt].rearrange("p (k c) -> p k c", c=512)
                if jt == 0:
                    for s in range(4):
                        cs = slice(s * 128, (s + 1) * 128)
                        nc.gpsimd.dma_start(xt[:, :, cs], xv[:, :, cs])
                else:
                    nc.gpsimd.dma_start(xt[:, 0:4, :], xv[:, 0:4, :])
                    nc.gpsimd.dma_start(xt[:, 4:8, :], xv[:, 4:8, :])
                return xt

            def dma_xt8(jt, queue):
                x8t = xtp.tile([128, KC, 512], F8, tag="xt8", bufs=3, name=f"x8t{jt}")
                x8v = x8p[jt].rearrange("p (k c) -> p k c", c=512)
                queue.dma_start(x8t[:, 0:4, :], x8v[:, 0:4, :])
                queue.dma_start(x8t[:, 4:8, :], x8v[:, 4:8, :])
                return x8t

            # Startup: the critical loads (pair-0 q/k weights + fp8 x chunk 0)
            # stream on the sync queue in consumption order; the gpsimd queue
            # is stalled behind a copy that depends on the last fp8 pair so
            # bulk loads don't contend with the critical path.
            dummy_in = consts.tile([1, 16], F32, tag="dummy_in")
            nc.vector.memset(dummy_in, 0.0)
            dummy_out = consts.tile([1, 16], F32, tag="dummy_out")
            nc.scalar.activation(dummy_out, dummy_in, AF.Exp)

            # PE warm-up: dummy matmuls with no DMA deps run during the
            # startup loads, pulling the tensor engine out of its low p-state
            # before real work lands.
            warm_w = consts.tile([64, 16], BT16, tag="warm_w")
            nc.vector.memset(warm_w, 0.5)
            warm_x = consts.tile([64, 512], BT16, tag="warm_x")
            nc.vector.memset(warm_x, 0.5)
            warm_ps = ps.tile([16, 512], F32, tag="fp", bufs=2, name="warm_ps")
            for wi in range(8):
                nc.tensor.matmul(warm_ps, warm_w, warm_x, start=True, stop=True)

            if QKPROJ_FP8:
                x8t0 = xtp.tile([128, KC, 512], F8, tag="xt8", bufs=3, name="x8t0")
                x8v0 = x8p[0].rearrange("p (k c) -> p k c", c=512)
                nc.sync.dma_start(wqk8_t[0][:, :, 0:2], wqk8_w[0][:, :, 0:2])
                nc.sync.dma_start(x8t0[:, 0:4, :], x8v0[:, 0:4, :])
                nc.sync.dma_start(wqk8_t[0][:, :, 2:4], wqk8_w[0][:, :, 2:4])
                nc.sync.dma_start(x8t0[:, 4:8, :], x8v0[:, 4:8, :])
                nc.sync.dma_start(wqk8_t[1], wqk8_w[1])
                nc.sync.dma_start(bq_sb2, bq)
                nc.sync.dma_start(bk_sb2, bk)
                nc.sync.dma_start(masks_sb[:, 0:1, :], masks[:, 0:1, :])
                nc.sync.dma_start(wqk8_t[2], wqk8_w[2])
                nc.sync.dma_start(masks_sb[:, 1:4, :], masks[:, 1:4, :])
                nc.sync.dma_start(wqk8_t[3], wqk8_w[3])
                x8ts[0] = x8t0
                # gpsimd stall: depends on the critical fp8 x chunk
                dummy_gp = consts.tile([1, 8], F8, tag="dummy_gp")
                nc.gpsimd.tensor_copy(dummy_gp, x8t0[0:1, 6, 0:8])
                xts = {0: dma_xt(0)}
            else:
                xt0 = xtp.tile([128, KC, 512], BT16, tag="xt", bufs=3, name="xt0")
                xv0 = xp[0].rearrange("p (k c) -> p k c", c=512)
                nc.sync.dma_start(xt0[:, 0:2, :], xv0[:, 0:2, :])
                nc.sync.dma_start(wqk_t[0], wqk_w[0])
                nc.sync.dma_start(xt0[:, 2:4, :], xv0[:, 2:4, :])
                nc.sync.dma_start(xt0[:, 4:6, :], xv0[:, 4:6, :])
                nc.sync.dma_start(xt0[:, 6:8, :], xv0[:, 6:8, :])
                nc.sync.dma_start(wqk_t[1], wqk_w[1])
                nc.sync.dma_start(masks_sb, masks)
                nc.sync.dma_start(wqk_t[2], wqk_w[2])
                nc.sync.dma_start(wqk_t[3], wqk_w[3])
                dummy_gp = consts.tile([1, 8], BT16, tag="dummy_gp")
                nc.gpsimd.tensor_copy(dummy_gp, xt0[0:1, 6, 0:8])
                xts = {0: xt0}

            # wv on the scalar queue, stalled behind the mid-critical fp8 x
            # pair so it overlaps (not contends with) the startup loads
            wv_sb = consts.tile([128, KC, HD], BT16, tag="wv")
            wv_v = wvp.rearrange("p (k c) -> p k c", c=512)
            dummy_sc = consts.tile([1, 8], F8 if QKPROJ_FP8 else BT16, tag="dummy_sc")
            if QKPROJ_FP8:
                nc.scalar.copy(dummy_sc, x8ts[0][0:1, 2, 0:8])
            else:
                nc.scalar.copy(dummy_sc, xts[0][0:1, 2, 0:8])
            nc.scalar.dma_start(wv_sb[:, 0:4, :], wv_v[:, 0:4, :])
            nc.scalar.dma_start(wv_sb[:, 4:8, :], wv_v[:, 4:8, :])
            wo_sb = consts.tile([128, NP, C], BT16, tag="wo")

            # ---- persistent activations ----
            qT = [qkp.tile([128, T], BT16, tag=f"qT{t}", name=f"qT{t}") for t in range(NP)]
            kT = [qkp.tile([128, T], BT16, tag=f"kT{t}", name=f"kT{t}") for t in range(NP)]
            v_sb = [vp.tile([128, HL * 65], BT16, tag=f"v{i}", name=f"v{i}") for i in range(NIK)]
            oT = [otp.tile([128, T], BT16, tag=f"oT{t}", name=f"oT{t}") for t in range(NP)]

            # ---- phase-1 units ----
            def qk_q(jt, t, xt):
                p = ps.tile([128, 512], F32, tag="fp", bufs=2, name=f"pq{jt}_{t}")
                if QKPROJ_FP8:
                    for j in range(KP):
                        nc.tensor.matmul(
                            p, wqk8_t[t][:, 0, j], xt[:, 2*j:2*j+2, :],
                            start=(j == 0), stop=(j == KP - 1),
                            perf_mode=(PM.DoubleRowSwInterleave if QK_SWIL
                                       else PM.DoubleRow),
                        )
                else:
                    for k in range(KC):
                        nc.tensor.matmul(
                            p, wqk_t[t][:, k, 0:128], xt[:, k, :],
                            start=(k == 0), stop=(k == KC - 1),
                        )
                nc.vector.tensor_scalar(
                    qT[t][:, jt * 512:(jt + 1) * 512], p,
                    0.125, bq_sb[:, t:t + 1], ALU.mult, ALU.add,
                )

            def qk_k(jt, t, xt):
                p = ps.tile([128, 512], F32, tag="fp", bufs=2, name=f"pk{jt}_{t}")
                if QKPROJ_FP8:
                    for j in range(KP):
                        nc.tensor.matmul(
                            p, wqk8_t[t][:, 1, j], xt[:, 2*j:2*j+2, :],
                            start=(j == 0), stop=(j == KP - 1),
                            perf_mode=(PM.DoubleRowSwInterleave if QK_SWIL
                                       else PM.DoubleRow),
                        )
                else:
                    for k in range(KC):
                        nc.tensor.matmul(
                            p, wqk_t[t][:, k, 128:256], xt[:, k, :],
                            start=(k == 0), stop=(k == KC - 1),
                        )
                nc.vector.tensor_scalar_add(
                    kT[t][:, jt * 512:(jt + 1) * 512], p, bk_sb[:, t:t + 1]
                )

            def v_unit(jt, s, xt):
                ik = jt * 4 + s
                p = ps.tile([128, 512], F32, tag="fp", bufs=2, name=f"pv{ik}")
                for k in range(KC):
                    nc.tensor.matmul(
                        p, xt[:, k, s * 128:(s + 1) * 128], wv_sb[:, k, :],
                        start=(k == 0), stop=(k == KC - 1),
                    )
                vg = v_sb[ik].rearrange("p (h c) -> p h c", c=65)
                nc.vector.tensor_copy(
                    vg[:, :, 0:64], p.rearrange("p (h c) -> p h c", c=64)
                )
                nc.gpsimd.memset(vg[:, :, 64:65], 1.0)

            # ---- out-projection (m, n) sub-chunk; one merged y DMA per m ----
            ys_tiles = {}

            def phase3_n(m, n, alt=False):
                p = ps.tile([128, 512], F32, tag="fp", bufs=2, name=f"py{m}_{n}")
                for t in range(NP):
                    nc.tensor.matmul(
                        p, oT[t][:, m * 128:(m + 1) * 128],
                        wo_sb[:, t, n * 512:(n + 1) * 512],
                        start=(t == 0), stop=(t == NP - 1),
                    )
                if n == 0:
                    ys_tiles[m] = ystp.tile([128, 1024], BT16, tag="y", name=f"ys{m}")
                ys = ys_tiles[m]
                if alt:
                    # tail: per-half writes on alternating queues so the final
                    # transfers start as soon as each half is evicted
                    nc.scalar.copy(ys[:, n * 512:(n + 1) * 512], p)
                    eng = nc.sync if m % 2 == 1 else nc.gpsimd
                    eng.dma_start(
                        y[m * 128:(m + 1) * 128, n * 512:(n + 1) * 512],
                        ys[:, n * 512:(n + 1) * 512],
                    )
                else:
                    nc.vector.tensor_copy(ys[:, n * 512:(n + 1) * 512], p)
                    if n == 1:
                        nc.gpsimd.dma_start(y[m * 128:(m + 1) * 128, :], ys)

            # ---- attention ----
            def st_block(t, jq, ik, pts):
                d = ik - 4 * jq
                c0 = 128 * d if d > 0 else 0   # first potentially-valid column
                st = ps.tile([128, 1024], F32, tag="st", name=f"st{t}_{jq}_{ik}")
                stg = st.rearrange("p (h q) -> p h q", q=512)
                for hh in range(2):
                    r = slice(hh * 64, hh * 64 + 64)
                    nc.tensor.matmul(
                        stg[:, hh, c0:512],
                        kT[t][r, ik * 128:(ik + 1) * 128],
                        qT[t][r, jq * 512 + c0:(jq + 1) * 512],
                        start=True, stop=True,
                    )
                pt = ptp.tile([128, 1024], BT16, tag="pt", name=f"pt{t}_{jq}_{ik}")
                ptg = pt.rearrange("p (h q) -> p h q", q=512)
                if d >= 0:
                    ptm = ptmpp.tile([128, 1024], BT16, tag="ptmp", name=f"ptm{t}_{jq}_{ik}")
                    ptmg = ptm.rearrange("p (h q) -> p h q", q=512)
                    nc.scalar.activation(ptmg[:, :, c0:512], stg[:, :, c0:512], AF.Exp)
                    for hh in range(2):
                        nc.vector.tensor_mul(
                            ptg[:, hh, c0:512],
                            ptmg[:, hh, c0:512],
                            masks_sb[:, d, c0:512],
                        )
                else:
                    nc.scalar.activation(pt, st, AF.Exp)
                pts[ik] = (pt, c0)

            def av(t, ik, nik, pts, o_ps):
                # split the K=128 contraction into row-group halves so every
                # attention matmul is a half-array op: the two groups stream
                # concurrently. Both halves accumulate into the same PSUM
                # region (FIFO start order makes the start=True half land
                # first at every address).
                pt, c0 = pts[ik]
                ptg = pt.rearrange("p (h q) -> p h q", q=512)
                for hh in range(2):
                    h = 2 * t + hh
                    if AV_HALVES:
                        for ha in range(2):
                            r = slice(64 * ha, 64 * ha + 64)
                            nc.tensor.matmul(
                                o_ps[hh][:, c0:512],
                                v_sb[ik][r, h * 65:h * 65 + 65],
                                ptg[r, hh, c0:512],
                                start=(ik == 0 and ha == 0),
                                stop=(ik == nik - 1 and ha == 1),
                                skip_group_check=True,
                            )
                    else:
                        nc.tensor.matmul(
                            o_ps[hh][:, c0:512], v_sb[ik][:, h * 65:h * 65 + 65],
                            ptg[:, hh, c0:512],
                            start=(ik == 0), stop=(ik == nik - 1),
                        )

            def attention(t, jq, fills, evict_split=False):
                nik = 4 * jq + 4
                o_ps = [
                    ps.tile([65, 512], F32, tag="ot", bufs=2, name=f"ops{t}_{jq}_{_h}")
                    for _h in range(2)
                ]
                pts = {}
                # homogeneous bursts: 4 st matmuls (2 ik x 2 row groups), one
                # fill chain, then 4 av chain matmuls for the previous pair
                for ik in range(nik):
                    st_block(t, jq, ik, pts)
                    if (jq == 0 and ik % 2 == 1) or (jq > 0 and ik % 4 == 3):
                        for _ in range(2):
                            if fills:
                                fills.popleft()()
                    if ik > 0:
                        av(t, ik - 1, nik, pts, o_ps)
                av(t, nik - 1, nik, pts, o_ps)
                # evict Z row + unnormalized O^T, freeing the PSUM accumulators.
                # Last unit: Z rows first so the tail 1/Z chain starts早.
                out_h = []
                if evict_split:
                    for hh in range(2):
                        ouz = znp.tile([65, 512], F32, tag="ouz", bufs=6, name=f"oz{t}_{jq}_{hh}")
                        nc.vector.tensor_copy(ouz[64:65, :], o_ps[hh][64:65, :])
                        out_h.append(ouz)
                    for hh in range(2):
                        nc.vector.tensor_copy(out_h[hh][0:64, :], o_ps[hh][0:64, :])
                else:
                    for hh in range(2):
                        ouz = znp.tile([65, 512], F32, tag="ouz", bufs=6, name=f"oz{t}_{jq}_{hh}")
                        nc.vector.tensor_copy(ouz, o_ps[hh])
                        out_h.append(ouz)
                while fills:
                    fills.popleft()()
                return out_h

            import concourse.bass as bass_mod

            def normalize_a1(t, jq, evicted):
                # Stage A1: gather both heads' Z rows [1,512] as [8,64] each
                # into one [16,64] tile. DMA only -- the dependent reciprocal
                # runs a unit later so no engine queue stalls on this latency.
                zb = znp.tile([16, 64], F32, tag="zb", bufs=3, name=f"zb{t}_{jq}")
                for hh in range(2):
                    ouz = evicted[hh]
                    nc.sync.dma_start(
                        zb[8 * hh:8 * hh + 8, :],
                        ouz[64:65, :].rearrange("o (p q) -> o p q", p=8),
                    )
                return zb

            def normalize_a2(t, jq, zb):
                # Stage A2: reciprocal, then broadcast 1/Z via a DRAM
                # round-trip (partition-step-0 DMA reads are legal from DRAM).
                rcp = znp.tile([16, 64], F32, tag="rcpb", bufs=2, name=f"rcp{t}_{jq}")
                nc.vector.reciprocal(rcp, zb)
                rcp16 = znp.tile([16, 64], BT16, tag="rcp16b", bufs=2, name=f"rcp16{t}_{jq}")
                nc.vector.tensor_copy(rcp16, rcp)
                nc.sync.dma_start(
                    rcp_dram[jq, 2 * t:2 * t + 2, :].rearrange("h (p q) -> (h p) q", p=8),
                    rcp16,
                )
                bcs = []
                for hh in range(2):
                    bc_sb = znp.tile([64, 512], BT16, tag="bc_sb", bufs=6, name=f"bs{t}_{jq}_{hh}")
                    src = rcp_dram[jq, 2 * t + hh, :]
                    bcast = bass_mod.AP(
                        tensor=src.tensor, offset=src.offset,
                        ap=[[0, 64]] + [list(a) for a in src.ap],
                    )
                    nc.sync.dma_start(bc_sb, bcast)
                    bcs.append(bc_sb)
                return bcs

            def normalize_b(t, jq, evicted, bcs):
                # Stage B (one slot later, after the broadcast landed): scale
                # O^T by 1/Z.
                qs2 = slice(jq * 512, (jq + 1) * 512)
                nc.vector.tensor_mul(oT[t][0:64, qs2], evicted[0][0:64, :], bcs[0])
                tmp = znp.tile([64, 512], BT16, tag="tmp_o", bufs=2, name=f"tm{t}_{jq}")
                nc.gpsimd.tensor_mul(tmp, evicted[1][0:64, :], bcs[1])
                nc.gpsimd.dma_start(oT[t][64:128, qs2], tmp)

            # ---- main schedule ----
            # unit u = jq*4 + t; prelude runs qk(0) and qk(1) so attention(u)
            # can fill qk(u+2) -- gives the v path two units of DMA headroom.
            def qk_unit(u):
                jt, tt = divmod(u, NP)
                xx = x8ts[jt] if QKPROJ_FP8 else xts[jt]
                qk_q(jt, tt, xx)
                qk_k(jt, tt, xx)

            pendA2 = deque()  # awaiting stage-A2 (rcp + broadcast), next unit
            pendB = deque()   # awaiting stage-B (muls), one slot later
            p3q = deque()
            qk_unit(0)
            if QKPROJ_FP8:
                x8ts[1] = dma_xt8(1, nc.sync)
            qk_unit(1)
            xts[1] = dma_xt(1)
            nc.gpsimd.dma_start(wo_sb, wo.rearrange("(t p) c -> p t c", p=128))
            last_ev = None
            for jq in range(NJQ):
                for t in range(NP):
                    u = jq * 4 + t
                    if t == 1 and jq >= 1 and jq + 1 < NJQ:
                        xts[jq + 1] = dma_xt(jq + 1)
                        if QKPROJ_FP8:
                            x8ts[jq + 1] = dma_xt8(jq + 1, nc.sync)
                    fills = deque()
                    if jq == 0 and t == 0:
                        for s in range(4):
                            fills.append(lambda ss=s: v_unit(0, ss, xts[0]))
                    if u + 2 < NJQ * NP:
                        fills.append(lambda uu=u + 2: qk_unit(uu))
                    # hold the late out-proj batches in p3q for the tail (they
                    # cover the 1/Z DRAM round-trip latency there)
                    if jq != NJQ - 1 and not (jq == NJQ - 2 and t >= 2):
                        for _ in range(2 if t == NP - 1 else 1):
                            if p3q:
                                m = p3q.popleft()
                                fills.append(lambda mm=m: phase3_n(mm, 0))
                                fills.append(lambda mm=m: phase3_n(mm, 1))
                    if t >= 2 and jq + 1 < NJQ:
                        s0 = 2 * (t - 2)
                        fills.append(lambda jt=jq + 1, s=s0: v_unit(jt, s, xts[jt]))
                        fills.append(lambda jt=jq + 1, s=s0 + 1: v_unit(jt, s, xts[jt]))
                    ev = attention(t, jq, fills,
                                   evict_split=(jq == NJQ - 1 and t == NP - 1))
                    if not (jq == NJQ - 1 and t == NP - 1):
                        zb = normalize_a1(t, jq, ev)
                        pendA2.append((t, jq, ev, zb))
                    while len(pendA2) >= 2:
                        ta2, ja2, eva2, zba2 = pendA2.popleft()
                        bcs = normalize_a2(ta2, ja2, zba2)
                        pendB.append((ta2, ja2, eva2, bcs))
                    nb_thresh = 2
                    while len(pendB) >= nb_thresh:
                        tb, jb, evb, bcsb = pendB.popleft()
                        normalize_b(tb, jb, evb, bcsb)
                        if tb == NP - 1:
                            p3q.extend(range(4 * jb, 4 * jb + 4))
                    last_ev = ev

            # ---- tail ----
            # (3,3): same DRAM-broadcast 1/Z path as every other unit. The
            # deferred out-proj chains in p3q keep the PE busy while the
            # broadcast lands; the final out-proj chains follow per m-chunk.
            ta, ja = NP - 1, NJQ - 1
            eva = last_ev
            zb_t = normalize_a1(ta, ja, eva)
            while pendA2:
                ta2, ja2, eva2, zba2 = pendA2.popleft()
                pendB.append((ta2, ja2, eva2, normalize_a2(ta2, ja2, zba2)))
            while pendB:
                tb, jb, evb, bcsb = pendB.popleft()
                normalize_b(tb, jb, evb, bcsb)
                if tb == NP - 1:
                    p3q.extend(range(4 * jb, 4 * jb + 4))
            bcs_t = normalize_a2(ta, ja, zb_t)
            while p3q:     # PE fill while the 1/Z chain completes
                m = p3q.popleft()
                phase3_n(m, 0)
                phase3_n(m, 1)
            for mi in range(4):
                cs = slice(mi * 128, (mi + 1) * 128)
                gs = slice(ja * 512 + mi * 128, ja * 512 + (mi + 1) * 128)
                nc.vector.tensor_mul(oT[ta][0:64, gs], eva[0][0:64, cs],
                                     bcs_t[0][:, cs])
                tmp = znp.tile([64, 128], BT16, tag="tmp_os", bufs=4, name=f"tms{mi}")
                nc.gpsimd.tensor_mul(tmp, eva[1][0:64, cs], bcs_t[1][:, cs])
                nc.sync.dma_start(oT[ta][64:128, gs], tmp)
                phase3_n(4 * ja + mi, 0, alt=True)
                phase3_n(4 * ja + mi, 1, alt=True)

    nc.compile()
    return nc


def _host_prep(x, wq, bq, wk, bk, wv, wo):
    masks_np = np.zeros((128, 4, 512), dtype=BF16)
    qn = np.arange(512)[None, :]
    kn = np.arange(128)[:, None]
    for d in range(4):
        masks_np[:, d, :] = (qn >= kn + 128 * d).astype(BF16)

    per_g = []
    for g in range(G):
        cs = slice(g * HD, (g + 1) * HD)
        wv_g = wv[:, cs]                                            # [C, 512]
        wvp = wv_g.reshape(KC, 128, 512).transpose(1, 0, 2).reshape(128, KC * 512)
        m = {
            "wvp": np.ascontiguousarray(wvp).astype(BF16),
            "wo": np.ascontiguousarray(wo[cs, :]).astype(BF16),
            "bq": np.ascontiguousarray((bq[cs] / 8.0).reshape(NP, 128).T).astype(np.float32),
            "bk": np.ascontiguousarray(bk[cs].reshape(NP, 128).T).astype(np.float32),
            "masks": masks_np,
        }
        if QKPROJ_FP8:
            # [p, t, pair, slab, m]: element = w[(2*pair+slab)*128 + p, g*HD + t*128 + m]
            def pack8(w):
                wg = w[:, cs].reshape(KC, 128, NP, 128)       # [k, p, t, m]
                wg = wg.reshape(KP, 2, 128, NP, 128)          # [pair, slab, p, t, m]
                wg = wg.transpose(2, 3, 0, 1, 4)              # [p, t, pair, slab, m]
                if QK_SWIL:
                    # HW SwInterleave layout: stored[:, 2c+s] = logical[:, s, 127-c]
                    rev = wg[..., ::-1]                       # [p,t,pair,s,c]
                    wg = rev.transpose(0, 1, 2, 4, 3)         # [p,t,pair,c,s]
                return np.ascontiguousarray(
                    wg.reshape(128, NP, KP * 256)).astype(FP8)
            m["wqk8"] = np.ascontiguousarray(np.stack(
                [pack8(wq), pack8(wk)], axis=2).reshape(128, NP, 2 * KP * 256))
        else:
            wqkp = np.empty((NP, 128, KC * 256), dtype=np.float32)
            for t in range(NP):
                ts_ = slice(g * HD + t * 128, g * HD + (t + 1) * 128)
                blk = np.concatenate([wq[:, ts_], wk[:, ts_]], axis=1)  # [C, 256]
                wqkp[t] = blk.reshape(KC, 128, 256).transpose(1, 0, 2).reshape(128, KC * 256)
            m["wqkp"] = np.ascontiguousarray(wqkp).astype(BF16)
        per_g.append(m)
    in_maps = []
    xps = []
    x8ps = []
    for b in range(B):
        xT = x[b].T                                                 # [C, T]
        xpb = (xT.reshape(KC, 128, NJQ, 512).transpose(2, 1, 0, 3)
               .reshape(NJQ, 128, KC * 512))
        xpb = np.ascontiguousarray(xpb)
        xps.append(xpb.astype(BF16))
        if QKPROJ_FP8:
            x8ps.append(xpb.astype(FP8))
    for c in range(8):
        b, g = divmod(c, G)
        m = dict(per_g[g])
        m["xp"] = xps[b]
        if QKPROJ_FP8:
            m["x8p"] = x8ps[b]
        in_maps.append(m)
    return in_maps


def kernel(x, wq, bq, wk, bk, wv, bv, wo, bo):
    x = np.asarray(x, dtype=np.float32)
    wq = np.asarray(wq, dtype=np.float32)
    bq = np.asarray(bq, dtype=np.float32)
    wk = np.asarray(wk, dtype=np.float32)
    bk = np.asarray(bk, dtype=np.float32)
    wv = np.asarray(wv, dtype=np.float32)
    bv = np.asarray(bv, dtype=np.float32)
    wo = np.asarray(wo, dtype=np.float32)
    bo = np.asarray(bo, dtype=np.float32)

    if "nc" not in _CACHED:
        _CACHED["nc"] = _build()
    nc = _CACHED["nc"]

    in_maps = _host_prep(x, wq, bq, wk, bk, wv, wo)
    res = run_bass_kernel_spmd(nc, in_maps, core_ids=list(range(8)))

    const_row = (bo.astype(np.float64) + bv.astype(np.float64) @ wo.astype(np.float64))
    out = np.empty((B, T, C), dtype=np.float32)
    for b in range(B):
        acc = res.results[2 * b]["y"].astype(np.float64)
        acc += res.results[2 * b + 1]["y"].astype(np.float64)
        acc += const_row[None, :]
        out[b] = acc.astype(np.float32)
    return out
